# revision 68
# baseline (speedup 1.0000x reference)
"""NeuronPool (moe_routing) Trainium2 kernel.

Expert-parallel over 8 NeuronCores: core c computes neurons [8c, 8c+8) for the
full batch, host concatenates along the neuron axis.

Per-core pipeline (all shapes per core):
  xT: 18 [128,32] bf16 chunks, hist-first.  The 16 hist chunks arrive
      pre-broadcast from the host in one DMA; the 2 projection chunks are
      computed on device (Wp.T @ emb.T + bp) from host-transposed emb.
  per neuron n:
      psum1[32,512] = sum_k xT[k].T @ W1[n,k]    (W1 fp8-e3m4 scaled x64 in
          HBM when representable; the 1/64 dequant rides gelu1's ACT scale)
      h1 = gelu(psum1) bf16 -> PE transpose + DVE copy -> h1T [128,32] x4
      psum2[32,512] = sum_j h1T[j].T @ W2[n,j]   (W2/W3 bf16 in HBM)
      h2 = gelu(psum2) -> transpose -> h2T
      psum3[32,256] = sum_j h2T[j].T @ W3[n,j]
      moments on ACT off PSUM: rs=sum(y), sq=sum(y^2)  (copy/square live in
          every ACT table set, so no mid-kernel table reloads)
      t = (y - mu)*gm in-loop on DVE (needs no 1/std)
  tail: ONE batched sqrt for all 8 neurons' 1/std (its sqrt-set table load
      is prefetched by a dummy sqrt right after the last gelu), 8 cheap
      per-neuron column scalings split across ACT/DVE, output in two DMAs.
Emission is software-pipelined one neuron deep with GEMM1(n+1)'s two halves
bracketing GEMM2(n), so the PE never head-of-line blocks on an ACT gelu.
Weight DMAs stream on the gpsimd queue in consumption order, small setup
DMAs on sync.  Biases/beta are all-zero for this model's initializer; the
build is specialized on that, verified at prep time (a general variant with
DVE bias adds is built and used if any is nonzero, and W1 falls back to
bf16 if its absmax overflows the scaled fp8 range).
Measured: ~78 us HW exec per core (baseline 170.7 us), rel err 1.34e-2
vs the fp32 reference (gate 2e-2), dominated by the fp8-e3m4 W1 rounding.
"""
import math
import numpy as np
import ml_dtypes
from contextlib import ExitStack

import concourse.bass as bass
import concourse.tile as tile
from concourse import bacc, mybir
from concourse.bass_utils import run_bass_kernel_spmd

N_CORES = 8
B = 32          # batch
D = 256         # model dim
HIST = 8
HID = 512
N_NEURONS = 64
NPC = N_NEURONS // N_CORES  # 8 neurons per core
IN_DIM = D * (1 + HIST)     # 2304
KC1 = IN_DIM // 128         # 18 contraction chunks for GEMM1
KC2 = HID // 128            # 4 chunks for GEMM2/GEMM3
LN_EPS = 1e-5
FMIN, FMAX = 0.5, 40.0
TICK_INTERVAL = 0.1
W1_SCALE = 64.0             # fp8 pre-scale; 1/W1_SCALE folds into gelu1
FP8_MAX = 15.5              # e3m4 max normal

f32 = mybir.dt.float32
f32r = mybir.dt.float32r
bf16 = mybir.dt.bfloat16
fp8 = mybir.dt.float8e3

_CACHE = {}


def _build_program(zbias, w1_fp8):
    """zbias: b1/b2/b3/beta all zero -> skip bias adds entirely.
    w1_fp8: W1 streams as fp8-e3m4 scaled by W1_SCALE."""
    nc = bacc.Bacc("TRN2", target_bir_lowering=False, debug=False,
                   num_devices=N_CORES)

    xetd = nc.dram_tensor("xetd", [128, 2, B], f32, kind="ExternalInput").ap()
    wp = nc.dram_tensor("wp", [D, D], f32, kind="ExternalInput").ap()
    bpd = nc.dram_tensor("bpd", [128, 2], f32, kind="ExternalInput").ap()
    histbd = nc.dram_tensor("histbd", [128, KC1 - 2, B], bf16,
                            kind="ExternalInput").ap()
    eyed = nc.dram_tensor("eyed", [32, 32], f32, kind="ExternalInput").ap()
    w1d = nc.dram_tensor("w1d", [NPC, 128, KC1, HID],
                         fp8 if w1_fp8 else bf16, kind="ExternalInput").ap()
    w2d = nc.dram_tensor("w2d", [NPC, 128, KC2, HID], bf16,
                         kind="ExternalInput").ap()
    w3d = nc.dram_tensor("w3d", [NPC, 128, KC2, D], bf16,
                         kind="ExternalInput").ap()
    # pre-broadcast per-neuron rows, replicated across the 32 batch
    # partitions on the host: [gm | (b1 b2 b3 bm when not zbias)]
    AUXW = D if zbias else D + HID + HID + D + D
    GM_OFF = 0
    B1_OFF, B2_OFF, B3_OFF, BM_OFF = D, D + HID, D + 2 * HID, 2 * D + 2 * HID
    auxd = nc.dram_tensor("auxd", [B, NPC, AUXW], f32, kind="ExternalInput").ap()
    out = nc.dram_tensor("out", [B, NPC, D], f32, kind="ExternalOutput").ap()

    GELU = mybir.ActivationFunctionType.Gelu
    COPY = mybir.ActivationFunctionType.Copy
    SQUARE = mybir.ActivationFunctionType.Square
    SQRT = mybir.ActivationFunctionType.Sqrt

    with tile.TileContext(nc) as tc, ExitStack() as ctx:
        # SBUF pools
        cst = ctx.enter_context(tc.tile_pool(name="cst", bufs=1))
        xtp = ctx.enter_context(tc.tile_pool(name="xtp", bufs=KC1))
        w1p = ctx.enter_context(tc.tile_pool(name="w1p", bufs=12))
        w23p = ctx.enter_context(tc.tile_pool(name="w23p", bufs=10))
        htp = ctx.enter_context(tc.tile_pool(name="htp", bufs=32))
        hp = ctx.enter_context(tc.tile_pool(name="hp", bufs=6))
        ysp = ctx.enter_context(tc.tile_pool(name="ysp", bufs=NPC))
        rsp = ctx.enter_context(tc.tile_pool(name="rsp", bufs=4))
        yp = ctx.enter_context(tc.tile_pool(name="yp", bufs=20 if zbias else 10))
        stp = ctx.enter_context(tc.tile_pool(name="stp", bufs=24 if zbias else 14))
        # PSUM pools (8 banks: 4 + 3, one spare)
        accp = ctx.enter_context(tc.tile_pool(name="accp", bufs=4, space="PSUM"))
        trp = ctx.enter_context(tc.tile_pool(name="trp", bufs=4, space="PSUM"))

        # ---- weight streaming on the gpsimd queue; small setup DMAs ride
        # the sync queue so neither blocks the other ----
        def dma_w1(n, fine=False):
            w1t = []
            if fine:  # neuron 0: smaller leading piece, triggered from the
                # DVE queue which boots ~1.5us before gpsimd, so the first
                # weight bytes land as early as possible
                lens = (5, 6, 7)
                off = 0
                for i, ln in enumerate(lens):
                    t = w1p.tile([128, ln, HID], w1d.dtype, tag="w1",
                                 name=f"w1f{off}")
                    nc.gpsimd.dma_start(out=t[:], in_=w1d[n][:, off:off + ln, :])
                    w1t.append(t)
                    off += ln
                return w1t, lens
            for s in range(2):
                t = w1p.tile([128, 9, HID], w1d.dtype, tag="w1")
                nc.gpsimd.dma_start(out=t[:], in_=w1d[n][:, 9 * s:9 * s + 9, :])
                w1t.append(t)
            return w1t, (9, 9)

        def dma_w2(n):
            w2t = w23p.tile([128, KC2, HID], bf16, tag="w23")
            nc.gpsimd.dma_start(out=w2t[:], in_=w2d[n])
            return w2t

        def dma_w3(n):
            w3t = w23p.tile([128, KC2, D], bf16, tag="w23")
            nc.gpsimd.dma_start(out=w3t[:], in_=w3d[n])
            return w3t

        # first weight bytes head the gpsimd queue
        w1ts, w2ts, w3ts, h1s = {}, {}, {}, {}
        w1ts[0] = dma_w1(0)

        # ---- constants ----
        eye = cst.tile([32, 32], f32, tag="eye")
        nc.sync.dma_start(out=eye[:], in_=eyed)
        epst = cst.tile([B, 1], f32, tag="epst")
        nc.vector.memset(epst[:], LN_EPS)

        # ---- x setup.  The x chunks are ordered hist-first (the host rolls
        # W1's k-chunks to match) and arrive pre-broadcast from the host in a
        # single DMA, so GEMM1(0)'s first half depends only on that DMA and
        # w1(0)a -- no on-device setup chain at all.
        xthist = cst.tile([128, KC1 - 2, B], bf16, tag="xthist")
        nc.sync.dma_start(out=xthist[:], in_=histbd)
        xT = [xthist[:, c, :] for c in range(KC1 - 2)]

        eyebf = cst.tile([32, 32], bf16, tag="eyebf")
        nc.vector.tensor_copy(eyebf[:], eye[:])

        def transpose4(h, width=HID):
            """PE transpose (bf16, 1 cycle/row) + DVE copy out of PSUM"""
            hT = []
            for j in range(width // 128):
                pt = trp.tile([128, 32], bf16, tag="tr")
                nc.tensor.transpose(pt[:], h[:, j * 128:(j + 1) * 128], eyebf[:])
                st = htp.tile([128, 32], bf16, tag="hT")
                nc.vector.tensor_copy(st[:], pt[:])
                hT.append(st)
            return hT

        p1s = {}

        def gemm1_half(n, half):
            # GEMM1 emitted in two halves so its matmuls can interleave with
            # neuron n-1's transposes/GEMM2 and cover the gelu latencies
            w1t, lens = w1ts[n]
            if half == 0:
                p1 = accp.tile([B, HID], f32, tag="acc", name=f"p1_{n}")
                p1s[n] = p1
            p1 = p1s[n]
            bounds = [0]
            for ln in lens:
                bounds.append(bounds[-1] + ln)
            ks = range(0, lens[0]) if half == 0 else range(lens[0], KC1)
            for k in ks:
                s = next(i for i in range(len(lens)) if bounds[i + 1] > k)
                nc.tensor.matmul(p1[:], xT[k], w1t[s][:, k - bounds[s], :],
                                 start=(k == 0), stop=(k == KC1 - 1))
            if half == 0:
                return None
            h1 = hp.tile([B, HID], bf16, tag="h")
            sc = (1.0 / W1_SCALE) if w1_fp8 else 1.0
            if zbias:
                nc.scalar.activation(h1[:], p1[:], GELU, scale=sc)
            else:
                hb = hp.tile([B, HID], f32, tag="hb")
                nc.vector.tensor_scalar_mul(hb[:], p1[:], sc)
                hc = hp.tile([B, HID], f32, tag="hb")
                nc.vector.tensor_add(hc[:], hb[:], b1t[n][:])
                nc.scalar.activation(h1[:], hc[:], GELU)
            return h1

        def gemm2(n, h1T):
            w2t = w2ts[n]
            p2 = accp.tile([B, HID], f32, tag="acc")
            for j in range(KC2):
                nc.tensor.matmul(p2[:], h1T[j][:], w2t[:, j, :],
                                 start=(j == 0), stop=(j == KC2 - 1))
            h2 = hp.tile([B, HID], bf16, tag="h")
            if zbias:
                nc.scalar.activation(h2[:], p2[:], GELU)
            else:
                hc = hp.tile([B, HID], f32, tag="hb")
                nc.vector.tensor_add(hc[:], p2[:], b2t[n][:])
                nc.scalar.activation(h2[:], hc[:], GELU)
            return h2

        ys = {}
        rs_all = cst.tile([B, NPC], f32, tag="rs_all")
        sq_all = cst.tile([B, NPC], f32, tag="sq_all")
        yo_all = cst.tile([B, NPC, D], f32, tag="yo_all")

        def gemm3(n, h2T):
            w3t = w3ts[n]
            p3 = accp.tile([B, D], f32, tag="acc")
            for j in range(KC2):
                nc.tensor.matmul(p3[:], h2T[j][:], w3t[:, j, :],
                                 start=(j == 0), stop=(j == KC2 - 1))

            # uncentered moments straight off PSUM (copy/square live in every
            # ACT table set, so these never force a table reload):
            #   rs = sum(y); sq = sum(y^2); var = sq/D - (rs/D)^2
            y = yp.tile([B, D], f32, tag="y")
            if zbias:
                nc.scalar.activation(y[:], p3[:], COPY,
                                     accum_out=rs_all[:, n:n + 1])
                sqs = yp.tile([B, D], f32, tag="y")
                nc.scalar.activation(sqs[:], p3[:], SQUARE,
                                     accum_out=sq_all[:, n:n + 1])
            else:
                yb = yp.tile([B, D], f32, tag="y")
                nc.vector.tensor_add(yb[:], p3[:], b3t[n][:])
                y = yp.tile([B, D], f32, tag="y", name=f"yc_{n}")
                nc.scalar.activation(y[:], yb[:], COPY,
                                     accum_out=rs_all[:, n:n + 1])
                sqs = yp.tile([B, D], f32, tag="y")
                nc.scalar.activation(sqs[:], yb[:], SQUARE,
                                     accum_out=sq_all[:, n:n + 1])
            # (y - mu)*gm does not need inv, so it runs here in-loop; the
            # tail only applies the batched 1/std column scale.  mu_n is
            # computed on ACT: the accumulator value only lands in rs_all via
            # ACT's trailing READ_ACCUMULATOR, so a DVE read here would race.
            mu_n = stp.tile([B, 1], f32, tag="st")
            nc.scalar.mul(mu_n[:], rs_all[:, n:n + 1], 1.0 / D)
            t = ysp.tile([B, D], f32, tag="ys")
            nc.vector.scalar_tensor_tensor(
                t[:], y[:], mu_n[:], gmt[n][:],
                mybir.AluOpType.subtract, mybir.AluOpType.mult)
            ys[n] = t

        def emit_B_final():
            # batched LN epilogue: ONE sqrt (one ACT_TABLE_LOAD after the
            # last gelu), 8 cheap column scalings, ONE output DMA
            mu = stp.tile([B, NPC], f32, tag="st")
            nc.scalar.mul(mu[:], rs_all[:], 1.0 / D)
            ey2 = stp.tile([B, NPC], f32, tag="st")
            nc.scalar.mul(ey2[:], sq_all[:], 1.0 / D)
            mu2 = stp.tile([B, NPC], f32, tag="st")
            nc.vector.tensor_tensor(mu2[:], mu[:], mu[:], mybir.AluOpType.mult)
            var = stp.tile([B, NPC], f32, tag="st")
            nc.vector.tensor_tensor(var[:], ey2[:], mu2[:],
                                    mybir.AluOpType.subtract)
            std = stp.tile([B, NPC], f32, tag="st")
            nc.scalar.activation(std[:], var[:], SQRT, bias=epst[:])
            inv = stp.tile([B, NPC], f32, tag="st")
            nc.vector.reciprocal(inv[:], std[:])
            for n in range(NPC):
                if zbias and n in (0, 2, 4):
                    nc.scalar.activation(yo_all[:, n, :], ys[n][:], COPY,
                                         scale=inv[:, n:n + 1])
                else:
                    nc.vector.tensor_scalar_mul(yo_all[:, n, :], ys[n][:],
                                                inv[:, n:n + 1])
                if not zbias:
                    nc.vector.tensor_add(yo_all[:, n, :], yo_all[:, n, :],
                                         bmt[n][:])
                if n == NPC // 2 - 1:
                    nc.sync.dma_start(out=out[:, 0:NPC // 2, :],
                                      in_=yo_all[:, 0:NPC // 2, :])
            nc.sync.dma_start(out=out[:, NPC // 2:, :],
                              in_=yo_all[:, NPC // 2:, :])

        # ---- prologue: GEMM1(0)'s hist half runs while the emb projection
        # chain (xe DMA -> transpose -> Wp matmul) fills chunks 16/17 ----
        gemm1_half(0, 0)

        bpt = cst.tile([128, 2], f32, tag="bpt")
        nc.sync.dma_start(out=bpt[:], in_=bpd)
        wpt = cst.tile([128, 2, D], f32r, tag="wpt")
        nc.gpsimd.dma_start(out=wpt[:], in_=wp.rearrange("(c p) d -> p c d", p=128))

        xet = cst.tile([128, 2, B], f32r, tag="xet")
        nc.gpsimd.dma_start(out=xet[:], in_=xetd)
        w2ts[0] = dma_w2(0)
        w3ts[0] = dma_w3(0)
        w1ts[1] = dma_w1(1)
        xeT = [xet[:, 0, :], xet[:, 1, :]]
        for m in range(2):
            pp = trp.tile([128, 32], f32, tag="tr")
            for k in range(2):
                nc.tensor.matmul(pp[:], wpt[:, k, m * 128:(m + 1) * 128], xeT[k],
                                 start=(k == 0), stop=(k == 1))
            xt = xtp.tile([128, 32], bf16, tag="xt")
            nc.vector.tensor_scalar_add(xt[:], pp[:], bpt[:, m:m + 1])
            xT.append(xt)

        # per-neuron broadcast rows as separate 2D tiles: 3D-sliced APs are
        # not reliable as tensor_tensor in1 operands on the DVE
        gmt, b1t, b2t, b3t, bmt = {}, {}, {}, {}, {}
        for n in range(NPC):
            gmt[n] = cst.tile([B, D], f32, tag=f"gmt{n}", name=f"gmt{n}")
            nc.scalar.dma_start(out=gmt[n][:], in_=auxd[:, n, GM_OFF:GM_OFF + D])
        if not zbias:
            for n in range(NPC):
                b1t[n] = cst.tile([B, HID], f32, tag=f"b1t{n}", name=f"b1t{n}")
                nc.sync.dma_start(out=b1t[n][:],
                                  in_=auxd[:, n, B1_OFF:B1_OFF + HID])
                b2t[n] = cst.tile([B, HID], f32, tag=f"b2t{n}", name=f"b2t{n}")
                nc.sync.dma_start(out=b2t[n][:],
                                  in_=auxd[:, n, B2_OFF:B2_OFF + HID])
                b3t[n] = cst.tile([B, D], f32, tag=f"b3t{n}", name=f"b3t{n}")
                nc.sync.dma_start(out=b3t[n][:],
                                  in_=auxd[:, n, B3_OFF:B3_OFF + D])
                bmt[n] = cst.tile([B, D], f32, tag=f"bmt{n}", name=f"bmt{n}")
                nc.sync.dma_start(out=bmt[n][:],
                                  in_=auxd[:, n, BM_OFF:BM_OFF + D])

        # ---- software pipeline, one neuron deep; GEMM1(n+1)'s two halves
        # bracket GEMM2(n) so the PE is never waiting on a gelu ----
        h1s[0] = gemm1_half(0, 1)
        for n in range(NPC):
            if n + 2 < NPC:
                w1ts[n + 2] = dma_w1(n + 2)
            if n + 1 < NPC:
                w2ts[n + 1] = dma_w2(n + 1)
                w3ts[n + 1] = dma_w3(n + 1)
                gemm1_half(n + 1, 0)
            h1T = transpose4(h1s[n])
            h2 = gemm2(n, h1T)
            if n == NPC - 1:
                # dummy sqrt: swaps the ACT table to the sqrt set right
                # after the final gelu, so the real batched sqrt in the
                # tail does not pay the ~1.3us ACT_TABLE_LOAD
                scr = stp.tile([B, 1], f32, tag="st")
                nc.scalar.activation(scr[:], epst[:], SQRT)
            if n + 1 < NPC:
                h1s[n + 1] = gemm1_half(n + 1, 1)
            h2T = transpose4(h2)
            gemm3(n, h2T)
        emit_B_final()

    nc.compile()
    return nc


def _get_program(zbias, w1_fp8):
    key = (zbias, w1_fp8)
    if key not in _CACHE:
        _CACHE[key] = _build_program(zbias, w1_fp8)
    return _CACHE[key]


def _prep_in_maps(input_embedding, pre_activations, Wp, bp, W1, b1, W2, b2, W3,
                  b3, gamma, beta, tick):
    emb = np.asarray(input_embedding, dtype=np.float32)
    hist = np.asarray(pre_activations, dtype=np.float32)
    Wp = np.asarray(Wp, dtype=np.float32)
    bp = np.asarray(bp, dtype=np.float32)
    W1 = np.asarray(W1, dtype=np.float32)
    b1 = np.asarray(b1, dtype=np.float32)
    W2 = np.asarray(W2, dtype=np.float32)
    b2 = np.asarray(b2, dtype=np.float32)
    W3 = np.asarray(W3, dtype=np.float32)
    b3 = np.asarray(b3, dtype=np.float32)
    gamma = np.asarray(gamma, dtype=np.float32)
    beta = np.asarray(beta, dtype=np.float32)

    zbias = (not b1.any()) and (not b2.any()) and (not b3.any()) \
        and (not beta.any())
    w1_fp8 = float(np.abs(W1).max()) * W1_SCALE <= FP8_MAX

    # oscillator modulation folded into gamma/beta
    i = np.arange(N_NEURONS, dtype=np.float64)
    freq = FMIN * (FMAX / FMIN) ** (i / (N_NEURONS - 1))
    phase = np.mod(i * 2.3571, 2.0 * math.pi)
    t = float(np.asarray(tick)) * TICK_INTERVAL
    mod = (1.0 + 0.5 * np.sin(2.0 * math.pi * freq * t + phase)).astype(np.float32)
    gm = (gamma * mod[:, None]).astype(np.float32)
    bm = (beta * mod[:, None]).astype(np.float32)

    # hist chunks pre-broadcast across the batch: histb[p, c, b] = hist_flat[128c+p]
    histb = np.ascontiguousarray(np.broadcast_to(
        hist.reshape(16, 128).T[:, :, None], (128, 16, B))).astype(
            ml_dtypes.bfloat16)
    # emb transposed into k-chunks: xet[p, k, b] = emb[b, 128k+p]
    xet = np.ascontiguousarray(emb.T.reshape(2, 128, B).transpose(1, 0, 2))
    bpd = np.ascontiguousarray(bp.reshape(2, 128).T)
    eyed = np.eye(32, dtype=np.float32)

    # weight layout: (n, p, k_chunk, hid) so each supertile DMA reads one
    # contiguous run per partition.  k-chunks are rolled so the hist rows
    # come first, matching the kernel's hist-first xT ordering.
    W1r = np.ascontiguousarray(
        np.roll(W1.reshape(N_NEURONS, KC1, 128, HID), -2, axis=1)
        .transpose(0, 2, 1, 3))
    if w1_fp8:
        W1r = (W1r * W1_SCALE).astype(ml_dtypes.float8_e3m4)
    else:
        W1r = W1r.astype(ml_dtypes.bfloat16)
    W2r = np.ascontiguousarray(
        W2.reshape(N_NEURONS, KC2, 128, HID).transpose(0, 2, 1, 3)).astype(
            ml_dtypes.bfloat16)
    W3r = np.ascontiguousarray(
        W3.reshape(N_NEURONS, KC2, 128, D).transpose(0, 2, 1, 3)).astype(
            ml_dtypes.bfloat16)

    # per-neuron rows pre-broadcast across the batch: [gm | b1 b2 b3 bm]
    if zbias:
        auxn = gm[:, None, :]                                  # (N, 1, D)
        auxn = np.broadcast_to(auxn, (N_NEURONS, B, D))        # (N, B, D)
    else:
        row = np.concatenate([gm, b1, b2, b3, bm], axis=1)
        auxn = np.broadcast_to(row[:, None, :],
                               (N_NEURONS, B, row.shape[1]))
    auxn = np.ascontiguousarray(auxn.transpose(1, 0, 2))       # (B, N, AUXW)

    in_maps = []
    for c in range(N_CORES):
        s = slice(c * NPC, (c + 1) * NPC)
        in_maps.append({
            "xetd": xet,
            "wp": Wp,
            "bpd": bpd,
            "histbd": histb,
            "eyed": eyed,
            "w1d": W1r[s],
            "w2d": W2r[s],
            "w3d": W3r[s],
            "auxd": np.ascontiguousarray(auxn[:, s, :]),
        })
    return in_maps, zbias, w1_fp8


def run(inputs, trace=False):
    in_maps, zbias, w1_fp8 = _prep_in_maps(**inputs)
    nc = _get_program(zbias, w1_fp8)
    br = run_bass_kernel_spmd(nc, in_maps, core_ids=list(range(N_CORES)),
                              trace=trace)
    out = np.concatenate([r["out"] for r in br.results], axis=1)
    return np.ascontiguousarray(out, dtype=np.float32), br


def kernel(**inputs) -> np.ndarray:
    out, _ = run(inputs, trace=False)
    return out
